# revision 24
# baseline (speedup 1.0000x reference)
"""CapsuleConv2d (3x3, stride 1, pad 1) with dynamic routing — Trainium2 Bass kernel.

Problem (hardcoded): x (4, 32, 56, 56) f32, weight (4, 4, 9, 8, 16) f32
  -> out (4, 64, 56, 56) f32.

Sharding: 8 cores = 4 batch x 2 pixel-halves of a zero-padded 58x58 grid.
Each core computes all (P_out, P_in) capsule groups for its half of the
padded pixel grid (7 super-tiles of 2x128 flat padded pixels); the host
unpads and stitches. Padding-garbage pixels are computed but discarded.

v2: fp16 end-to-end.
  PE    : fp16 matmuls (1 cycle/row vs 4 for f32): per-tap priors into a
          shared PSUM slot, plus the tap-sum s0 via 3 vertically-stacked
          (96-partition) matmuls against dk-stacked weights.
  ACT   : copies priors PSUM->SBUF fp16 (k-innermost layout), squares,
          Ln/Exp squash scalars, softmax Exp.
  DVE   : fp16 2x-mode products (P x probs / P x v broadcasts) and fp16
          pairwise-tree reductions; f32 only for tiny ops.
  GPSIMD: one product pass + one big reduce + assorted small reduces,
          balancing the three vector engines.
  Routing algebra: outputs o_i are never materialized; the squash scale
  g_i is folded into the next logits update (l += g * (P . v)).
"""

import sys

sys.path.insert(0, "/opt/trn_rl_repo")

import numpy as np

import concourse.bacc as bacc
import concourse.mybir as mybir
from concourse.bass_utils import run_bass_kernel_spmd
from concourse.hw_specs import get_activation_tables
from concourse.tile import TileContext

# All ACT funcs used here (Square, Ln, Exp) live in act table 6
# ("natural_log_exp_and_others"); pin it so the table loads once.
_ACT_TABLE_NAME = "natural_log_exp_and_others"


class _PinnedActBacc(bacc.Bacc):
    def insert_act_table_loads(self):
        tabs = get_activation_tables(self.m.arch)
        names = list(tabs.keys())
        idx = names.index(_ACT_TABLE_NAME)
        only = [(_ACT_TABLE_NAME, tabs[_ACT_TABLE_NAME])]
        bacc._bass_rust.insert_act_table_loads(self, only)
        for bb in self.main_func.blocks:
            for inst in bb.instructions:
                if type(inst).__name__ == "InstLoadActFuncSet":
                    if inst.act_func_set_id != idx:
                        inst.act_func_set_id = idx


F32 = mybir.dt.float32
F16 = mybir.dt.float16
AF = mybir.ActivationFunctionType
ALU = mybir.AluOpType
AX = mybir.AxisListType

# geometry
PIN, LIN, POUT, LOUT, KK = 4, 8, 4, 16, 9
CIN = PIN * LIN          # 32
OPD = POUT * PIN * LOUT  # 256 free cols per tap
NGRP = POUT * PIN        # 16 (o,p) squash groups per block
HP = 58                  # padded grid side
NPIX = HP * HP           # 3364 padded pixels
TILE = 128
NB = 2                   # pixel blocks fused per super-tile
NST = 7                  # super-tiles per core
CORE_PIX = NST * NB * TILE   # 1792
P0_B = NPIX - CORE_PIX   # 1572: second half start
NCH = POUT * LOUT        # 64 output channels

# xin layout (fp16, 128 partitions):
#   partitions 0..95 : x3 = 3 vertical-tap bands of the padded x window,
#                      band dj at columns shifted by (dj-1)*HP.
#                      cols [0, X3W): block t tap (dj,dk) reads
#                      x3[dj*32:(dj+1)*32, t*TILE + dk : t*TILE + dk + TILE]
#                      then ws0 = 3 dk-stacked s0 stationaries [96, 3*OPD]
#   partitions 96..127: wm = per-tap moving weights [32, KK*OPD]
X3W = CORE_PIX + 2       # 1794
WS0_OFF = X3W            # 3 dk-stacked s0 stationaries [96, 3*OPD]
WM_OFF = X3W + 3 * OPD   # per-tap moving weights, replicated per band
XIN_COLS = WM_OFF + KK * OPD  # 4866


def build_program():
    nc = _PinnedActBacc("TRN2", target_bir_lowering=False)
    xin_d = nc.dram_tensor("xin", [96, XIN_COLS], F16, kind="ExternalInput")
    out_d = nc.dram_tensor("out", [CORE_PIX, NCH], F32, kind="ExternalOutput")

    with TileContext(nc) as tc:
        with (
            tc.tile_pool(name="const", bufs=1) as const,
            tc.tile_pool(name="pbig", bufs=1) as pbig,
            tc.tile_pool(name="tbig", bufs=1) as tbig,
            tc.tile_pool(name="small", bufs=2) as small,
            tc.tile_pool(name="outp", bufs=2) as outp,
            tc.tile_pool(name="psum_p", bufs=1, space="PSUM") as psum_p,
            tc.tile_pool(name="psum_s", bufs=1, space="PSUM") as psum_s,
        ):
            xin = const.tile([96, XIN_COLS], F16)
            nchunk = 4
            cs = (XIN_COLS + nchunk - 1) // nchunk
            for ci in range(nchunk):
                lo, hi = ci * cs, min((ci + 1) * cs, XIN_COLS)
                nc.sync.dma_start(out=xin[:, lo:hi], in_=xin_d[:, lo:hi])
            x3 = xin[:, :X3W]
            ws0 = xin[:, WS0_OFF:WS0_OFF + 3 * OPD]
            eps_t = const.tile([TILE, 1], F32, tag="eps")
            nc.vector.memset(eps_t, 1e-30)
            one_t = const.tile([TILE, 1], F32, tag="one")
            nc.vector.memset(one_t, 1.0)

            NG = NB * NGRP  # 32 squash groups across blocks

            def squash_g(sq, sfx):
                """g = sqrt(u)/(1+u) from u=|s|^2, via
                exp(0.5*ln(u+eps) - ln(u+1)). Returns g [TILE, NG] f32."""
                la = small.tile([TILE, NG], F32, tag="la" + sfx)
                nc.scalar.activation(out=la, in_=sq, func=AF.Ln, bias=eps_t[:, :])
                lb = small.tile([TILE, NG], F32, tag="lb" + sfx)
                nc.scalar.activation(out=lb, in_=sq, func=AF.Ln, bias=one_t[:, :])
                yield
                cc = small.tile([TILE, NG], F32, tag="cc" + sfx)
                nc.vector.scalar_tensor_tensor(
                    out=cc, in0=la, scalar=0.5, in1=lb,
                    op0=ALU.mult, op1=ALU.subtract,
                )
                g = small.tile([TILE, NG], F32, tag="g" + sfx)
                nc.scalar.activation(out=g, in_=cc, func=AF.Exp)
                yield
                return g

            def sq_of(v16, eng, sfx):
                """|v|^2 per (b, g): ACT square + grouped d-reduce."""
                v2 = small.tile([TILE, NB, NGRP, LOUT], F32, tag="v2" + sfx)
                nc.scalar.activation(out=v2, in_=v16, func=AF.Square)
                yield
                sq = small.tile([TILE, NG], F32, tag="sq" + sfx)
                eng.tensor_reduce(
                    out=sq.rearrange("p (b g) -> p b g", b=NB),
                    in_=v2, axis=AX.X, op=ALU.add,
                )
                yield
                return sq

            def softmax_k(lg16, sfx):
                """softmax over k of fp16 logits [TILE, NB, KK, NGRP] ->
                probs fp16 [TILE, NB, NGRP, KK] (k innermost, packed)."""
                e16 = small.tile([TILE, NB, NGRP, KK], F32, tag="e" + sfx)
                nc.scalar.activation(
                    out=e16, in_=lg16.rearrange("p b k g -> p b g k"), func=AF.Exp
                )
                yield
                z = small.tile([TILE, NG], F32, tag="z" + sfx)
                nc.vector.tensor_reduce(
                    out=z.rearrange("p (b g) -> p b g", b=NB),
                    in_=e16, axis=AX.X, op=ALU.add,
                )
                yield
                zr = small.tile([TILE, NG], F32, tag="zr" + sfx)
                nc.vector.reciprocal(out=zr, in_=z)
                yield
                pr = small.tile([TILE, NB, NGRP, KK], F16, tag="pr" + sfx)
                nc.vector.tensor_mul(
                    pr, e16,
                    zr.rearrange("p (b g) -> p b g", b=NB)
                    .unsqueeze(3).to_broadcast([TILE, NB, NGRP, KK]),
                )
                yield
                return pr

            def weighted_v(psbK, pr, sfx):
                """v = sum_k pr_k * P_k -> fp16 [TILE, NB, NGRP, LOUT].
                Product in DVE 2x (k innermost both operands), then a
                contiguous-split pairwise k-tree."""
                t = tbig.tile([TILE, NB, NGRP, LOUT, KK], F16, tag="t" + sfx)
                nc.vector.tensor_mul(
                    t, psbK,
                    pr.unsqueeze(3).to_broadcast([TILE, NB, NGRP, LOUT, KK]),
                )
                yield
                u1 = tbig.tile([TILE, NB, NGRP, LOUT, 4], F16, tag="w1" + sfx)
                nc.vector.tensor_add(u1, t[:, :, :, :, 0:4], t[:, :, :, :, 4:8])
                yield
                u2 = tbig.tile([TILE, NB, NGRP, LOUT, 2], F16, tag="w2" + sfx)
                nc.vector.tensor_add(u2, u1[:, :, :, :, 0:2], u1[:, :, :, :, 2:4])
                yield
                u3 = small.tile([TILE, NB, NGRP, LOUT], F16, tag="u3" + sfx)
                nc.vector.tensor_add(u3, u2[:, :, :, :, 0], u2[:, :, :, :, 1])
                yield
                v16 = small.tile([TILE, NB, NGRP, LOUT], F16, tag="v" + sfx)
                nc.vector.tensor_add(v16, u3, t[:, :, :, :, 8])
                yield
                return v16

            def logits_pass(psbP, v16, g, lprev16, prod_eng, sfx):
                """l_new = lprev + g * (P . v): product (GPSIMD reads the
                k-innermost priors via a strided view at no cost; DVE needs
                the d-innermost psbD copy for 2x), fp16 d-tree on DVE, small
                scale/add. Returns fp16 [TILE, NB, KK, NGRP]."""
                t = tbig.tile([TILE, NB, KK, NGRP, LOUT], F16, tag="t" + sfx)
                prod_eng.tensor_mul(
                    t, psbP,
                    v16.unsqueeze(2).to_broadcast([TILE, NB, KK, NGRP, LOUT]),
                )
                yield
                w1 = tbig.tile([TILE, NB, KK, NGRP, 8], F16, tag="w1" + sfx)
                nc.vector.tensor_add(w1, t[..., 0:8], t[..., 8:16])
                yield
                w2 = tbig.tile([TILE, NB, KK, NGRP, 4], F16, tag="w2" + sfx)
                nc.vector.tensor_add(w2, w1[..., 0:4], w1[..., 4:8])
                yield
                w3 = tbig.tile([TILE, NB, KK, NGRP, 2], F16, tag="w3" + sfx)
                nc.vector.tensor_add(w3, w2[..., 0:2], w2[..., 2:4])
                yield
                lr = small.tile([TILE, NB, KK, NGRP], F32, tag="lr" + sfx)
                nc.vector.tensor_reduce(out=lr, in_=w3, axis=AX.X, op=ALU.add)
                yield
                lnew = small.tile([TILE, NB, KK, NGRP], F16, tag="ln" + sfx)
                if lprev16 is None:
                    nc.vector.tensor_mul(
                        lnew, lr,
                        g.rearrange("p (b g) -> p b g", b=NB)
                        .unsqueeze(2).to_broadcast([TILE, NB, KK, NGRP]),
                    )
                    yield
                else:
                    gl = small.tile([TILE, NB, KK, NGRP], F16, tag="gl" + sfx)
                    nc.gpsimd.tensor_mul(
                        gl, lr,
                        g.rearrange("p (b g) -> p b g", b=NB)
                        .unsqueeze(2).to_broadcast([TILE, NB, KK, NGRP]),
                    )
                    yield
                    nc.vector.tensor_add(lnew, gl, lprev16)
                    yield
                return lnew

            def tile_body(st, sfx):
                # ---- tap-sum s0 for both blocks (3-band stacked matmuls) --
                s0 = psum_s.tile([TILE, NB, OPD], F32, tag="s0" + sfx)
                for b in range(NB):
                    t = st * NB + b
                    for dk in range(3):
                        nc.tensor.matmul(
                            s0[:, b],
                            x3[:, t * TILE + dk:t * TILE + dk + TILE],
                            ws0[:, dk * OPD:(dk + 1) * OPD],
                            start=(dk == 0), stop=(dk == 2),
                        )
                        yield
                # s016 = s0/9 in fp16 (o0 = g0 * s016 later, folded)
                s016 = small.tile([TILE, NB, NGRP, LOUT], F16, tag="s016" + sfx)
                sq0 = small.tile([TILE, NG], F32, tag="sq0" + sfx)
                for b in range(NB):
                    nc.scalar.activation(
                        out=s016[:, b].rearrange("p g d -> p (g d)"),
                        in_=s0[:, b], func=AF.Copy, scale=1.0 / 9.0,
                    )
                    yield
                # |s016|^2 via ACT square + GPSIMD grouped reduce
                v20 = small.tile([TILE, NB, NGRP, LOUT], F32, tag="v20" + sfx)
                nc.scalar.activation(out=v20, in_=s016, func=AF.Square)
                yield
                nc.vector.tensor_reduce(
                    out=sq0.rearrange("p (b g) -> p b g", b=NB),
                    in_=v20, axis=AX.X, op=ALU.add,
                )
                yield

                # ---- per-tap priors, block by block through the shared PSUM
                # slot; ACT copies each block to SBUF fp16 (k innermost) ----
                psbK = pbig.tile([TILE, NB, NGRP, LOUT, KK], F16, tag="pk" + sfx)
                for b in range(NB):
                    t = st * NB + b
                    pp = psum_p.tile([TILE, KK, OPD], F32, tag="pp")
                    for k in range(KK):
                        dj, dk = divmod(k, 3)
                        nc.tensor.matmul(
                            pp[:, k, :],
                            x3[dj * 32:(dj + 1) * 32,
                               t * TILE + dk:t * TILE + dk + TILE],
                            xin[dj * 32:(dj + 1) * 32,
                                WM_OFF + k * OPD:WM_OFF + (k + 1) * OPD],
                            start=True, stop=True,
                        )
                        yield
                    nc.scalar.copy(
                        out=psbK[:, b].rearrange("p g d k -> p k (g d)"),
                        in_=pp,
                    )
                    yield

                # k-outer fp16 copy for the DVE 2x logits product (on ACT:
                # DVE is the bottleneck engine, ACT has slack)
                psbD = pbig.tile([TILE, NB, KK, NGRP, LOUT], F16, tag="pd" + sfx)
                nc.scalar.copy(
                    out=psbD,
                    in_=psbK.rearrange("p b g d k -> p b k g d"),
                )
                yield

                # ---- iter 0: g0 from |s0/9|^2; l1 = g0 * (P . s016) ----
                g0 = yield from squash_g(sq0, sfx)
                l1 = yield from logits_pass(
                    psbK.rearrange("p b g d k -> p b k g d"), s016, g0, None,
                    nc.gpsimd, sfx,
                )
                # ---- iter 1: probs1, v1, g1, l2 = l1 + g1 * (P . v1) ----
                pr1 = yield from softmax_k(l1, sfx)
                v1 = yield from weighted_v(psbK, pr1, sfx)
                sq1 = yield from sq_of(v1, nc.vector, sfx)
                g1 = yield from squash_g(sq1, sfx)
                l2 = yield from logits_pass(psbD, v1, g1, l1, nc.vector, sfx)
                # ---- iter 2: probs2, v2, g2; out = sum_p g2 * v2 ----
                pr2 = yield from softmax_k(l2, sfx)
                v2 = yield from weighted_v(psbK, pr2, sfx)
                sq2 = yield from sq_of(v2, nc.vector, sfx)
                g2 = yield from squash_g(sq2, sfx)
                o2 = small.tile([TILE, NB, NGRP, LOUT], F32, tag="o2" + sfx)
                nc.gpsimd.tensor_mul(
                    o2, v2,
                    g2.rearrange("p (b g) -> p b g", b=NB)
                    .unsqueeze(3).to_broadcast([TILE, NB, NGRP, LOUT]),
                )
                yield
                # sum over input planes p: [b, (o q) d] -> [b, o d]
                r = outp.tile([TILE, NB, NCH], F32, tag="rr" + sfx)
                nc.vector.tensor_reduce(
                    out=r.rearrange("p b (o d) -> p b o d", o=POUT),
                    in_=o2.rearrange("p b (o q) d -> p b o d q", o=POUT),
                    axis=AX.X, op=ALU.add,
                )
                yield
                nc.sync.dma_start(
                    out=out_d[st * NB * TILE:(st + 1) * NB * TILE, :]
                    .rearrange("(b p) c -> p b c", b=NB),
                    in_=r,
                )

            # Interleave instruction emission with a sliding window of two
            # super-tiles so each engine's in-order queue alternates between
            # independent dependency chains.
            gens = []
            nxt = 0
            while gens or nxt < NST:
                while len(gens) < 3 and nxt < NST:
                    gens.append(tile_body(nxt, "ABC"[nxt % 3]))
                    nxt += 1
                for gn in list(gens):
                    try:
                        next(gn)
                    except StopIteration:
                        gens.remove(gn)
    nc.compile()
    return nc


_PROG = None


def _get_prog():
    global _PROG
    if _PROG is None:
        _PROG = build_program()
    return _PROG


def _make_inputs(x, weight):
    # moving weights for per-tap priors: block-diagonal [c=(p,l), (k, o, p, d)]
    wmov = np.zeros((CIN, KK, POUT, PIN, LOUT), np.float32)
    for p in range(PIN):
        wmov[p * LIN:(p + 1) * LIN, :, :, p, :] = np.transpose(
            weight[:, p], (2, 1, 0, 3)
        )  # (l, k, o, d) from (o, k, l, d)
    wm = wmov.reshape(CIN, KK * OPD).astype(np.float16)

    # s0 stationaries: ws0[dk] = dj-stacked [96=(dj,c), OPD]
    ws0 = np.zeros((3, 3, CIN, POUT, PIN, LOUT), np.float32)
    for dj in range(3):
        for dk in range(3):
            ws0[dk, dj] = wmov[:, dj * 3 + dk]
    ws0 = ws0.reshape(3, 96, OPD).transpose(1, 0, 2).reshape(96, 3 * OPD)
    ws0 = ws0.astype(np.float16)

    xp = np.pad(x, ((0, 0), (0, 0), (1, 1), (1, 1))).reshape(4, CIN, NPIX)
    xpm = np.pad(xp, ((0, 0), (0, 0), (64, 64))).astype(np.float16)
    in_maps = []
    for c in range(8):
        n, half = divmod(c, 2)
        p0 = 0 if half == 0 else P0_B
        xin = np.zeros((96, XIN_COLS), np.float16)
        # x3 bands: band dj covers pixel p0-1+(dj-1)*HP + [0, X3W)
        for dj in range(3):
            lo = 64 + p0 - 1 + (dj - 1) * HP
            xin[dj * 32:(dj + 1) * 32, :X3W] = xpm[n][:, lo:lo + X3W]
            xin[dj * 32:(dj + 1) * 32, WM_OFF:] = wm
        xin[:, WS0_OFF:WS0_OFF + 3 * OPD] = ws0
        in_maps.append({"xin": xin})
    return in_maps


def _assemble(results):
    out = np.empty((4, NCH, 56, 56), np.float32)
    for n in range(4):
        full = np.empty((NCH, NPIX), np.float32)
        full[:, :CORE_PIX] = results[2 * n]["out"].T
        full[:, CORE_PIX:] = results[2 * n + 1]["out"].T[:, CORE_PIX - P0_B:]
        out[n] = full.reshape(NCH, HP, HP)[:, 1:57, 1:57]
    return out


def kernel(x, weight):
    x = np.asarray(x, np.float32)
    weight = np.asarray(weight, np.float32)
    in_maps = _make_inputs(x, weight)
    last_err = None
    for _ in range(3):  # retry transient NRT/device errors
        try:
            res = run_bass_kernel_spmd(
                _get_prog(), in_maps, core_ids=list(range(8))
            )
            return _assemble(res.results)
        except Exception as e:  # noqa: BLE001
            last_err = e
    raise last_err


if __name__ == "__main__":
    rng = np.random.default_rng(0)
    x = rng.standard_normal((4, 32, 56, 56), dtype=np.float32)
    w = rng.standard_normal((4, 4, 9, 8, 16), dtype=np.float32)
    y = kernel(x, w)
    print("out", y.shape, y.dtype, float(np.abs(y).mean()))
